# revision 6
# baseline (speedup 1.0000x reference)
"""CLIPMaskedSpatialViT on 8 Trainium2 NeuronCores.

Strategy: data-parallel over batch (B=8 -> 1 image/core, weights replicated).
Per core the whole model runs feature-major ([D, tokens] with D on SBUF
partitions, 6 chunks of 128). The 16 per-mask global tokens (gv) are carried
as 16 extra columns next to the 197 x-tokens (213 columns total), so LN /
QKV / MLP / projections for the gv stream ride in the same matmuls as the
x stream. All matmuls are bf16 with fp32 PSUM accumulation:
  - LN affine folded into the following matmul's weights (host-side)
  - biases applied as K=1 matmul rows (lhsT = bias row, rhs = ones row)
  - 1/sqrt(hd) folded into Q weights
  - softmax without max-subtraction (scores are O(5)); attention denominators
    come out of the AV matmul via a ones-column appended to each head's V
  - attention masks applied multiplicatively (0/1) on exp(scores)
  - QuickGELU as h * ACT-Sigmoid(1.702 h)
"""
import sys

for _p in ("/opt/trn_rl_repo", "/root/.axon_site/_ro/trn_rl_repo"):
    if _p not in sys.path:
        sys.path.append(_p)

import numpy as np
import ml_dtypes

import concourse.bass as bass
import concourse.bacc as bacc
import concourse.mybir as mybir
import concourse.tile as tile
from concourse.bass_utils import run_bass_kernel_spmd

L, D, NH, FF, GRID, OUT = 12, 768, 12, 3072, 14, 512
T = GRID * GRID + 1          # 197 x tokens
NM = 16                      # masks == gv tokens
TC = T + NM                  # 213 combined columns
DC = D // 128                # 6
QK = 2 * D                   # 1536
QKC = QK // 128              # 12
FFC = FF // 128              # 24
HD = D // NH                 # 64
VW = NH * (HD + 1)           # 780 (per-head 64 V cols + 1 ones col)
NCORES = 8
EPS = 1e-5

BF = ml_dtypes.bfloat16
F32 = mybir.dt.float32
BF16 = mybir.dt.bfloat16
AF = mybir.ActivationFunctionType
ALU = mybir.AluOpType

# packed bias offsets: [bqk | bv' | bo | bfc | bcp]
OB_QK, OB_V, OB_O, OB_FC, OB_CP = 0, QK, QK + VW, QK + VW + D, QK + VW + D + FF
BIASW = QK + VW + D + FF + D   # 6924


def _build_nc():
    nc = bacc.Bacc("TRN2", target_bir_lowering=False, debug=False)

    xpat_d = nc.dram_tensor("xpat", [DC, 128, T - 1], BF16, kind="ExternalInput").ap()
    post_d = nc.dram_tensor("post", [DC, 128, T], F32, kind="ExternalInput").ap()
    lnpre_d = nc.dram_tensor("lnpre", [DC, 128, 2], F32, kind="ExternalInput").ap()
    wconv_d = nc.dram_tensor("wconv", [DC, 128, D], BF16, kind="ExternalInput").ap()
    wqk_d = nc.dram_tensor("wqk", [L, DC, 128, QK], BF16, kind="ExternalInput").ap()
    wv_d = nc.dram_tensor("wv", [L, DC, 128, VW], BF16, kind="ExternalInput").ap()
    wo_d = nc.dram_tensor("wo", [L, DC, 128, D], BF16, kind="ExternalInput").ap()
    wfc_d = nc.dram_tensor("wfc", [L, DC, 128, FF], BF16, kind="ExternalInput").ap()
    wcp_d = nc.dram_tensor("wcp", [L, FFC, 128, D], BF16, kind="ExternalInput").ap()
    bias_d = nc.dram_tensor("biases", [L, 1, BIASW], BF16, kind="ExternalInput").ap()
    wproj_d = nc.dram_tensor("wproj", [DC, 128, OUT], BF16, kind="ExternalInput").ap()
    mk0_d = nc.dram_tensor("mk0", [128, NM], BF16, kind="ExternalInput").ap()
    mk1_d = nc.dram_tensor("mk1", [85, NM], BF16, kind="ExternalInput").ap()
    y_d = nc.dram_tensor("y", [4, 128, NM], F32, kind="ExternalOutput").ap()

    with tile.TileContext(nc) as tc:
        _emit(nc, tc, xpat_d, post_d, lnpre_d, wconv_d, wqk_d, wv_d, wo_d,
              wfc_d, wcp_d, bias_d, wproj_d, mk0_d, mk1_d, y_d)
    nc.finalize()
    return nc


def _emit(nc, tc, xpat_d, post_d, lnpre_d, wconv_d, wqk_d, wv_d, wo_d,
          wfc_d, wcp_d, bias_d, wproj_d, mk0_d, mk1_d, y_d):
    from contextlib import ExitStack
    ctx = ExitStack()
    with ctx:
        pers = ctx.enter_context(tc.tile_pool(name="pers", bufs=1))
        act = ctx.enter_context(tc.tile_pool(name="act", bufs=2))
        wpool = ctx.enter_context(tc.tile_pool(name="w", bufs=8))
        wfcp = ctx.enter_context(tc.tile_pool(name="wfc", bufs=6))
        biasp = ctx.enter_context(tc.tile_pool(name="biasp", bufs=1))
        gtp = ctx.enter_context(tc.tile_pool(name="gtp", bufs=1))
        small = ctx.enter_context(tc.tile_pool(name="small", bufs=2))
        pmm = ctx.enter_context(tc.tile_pool(name="pmm", bufs=6, space="PSUM"))
        ppv = ctx.enter_context(tc.tile_pool(name="ppv", bufs=1, space="PSUM"))

        # ---- constants / persistent ----
        xT = pers.tile([128, DC, TC], F32, tag="xT")          # residual stream
        ones_t = pers.tile([1, 512], BF16, tag="ones")
        nc.vector.memset(ones_t[:], 1.0)
        stat_lhs = pers.tile([128, 1], BF16, tag="statl")
        nc.vector.memset(stat_lhs[:], 1.0)
        eps_t = pers.tile([1, 1], F32, tag="eps")
        nc.vector.memset(eps_t[:], EPS)
        lnpre_t = pers.tile([128, DC, 2], F32, tag="lnpre")
        nc.sync.dma_start(lnpre_t[:], lnpre_d.rearrange("c p two -> p c two"))
        mk0_t = pers.tile([128, NM], BF16, tag="mk0")
        nc.sync.dma_start(mk0_t[:], mk0_d)
        mk1_t = pers.tile([85, NM], BF16, tag="mk1")
        nc.sync.dma_start(mk1_t[:], mk1_d)

        # ---- LN helper ----
        def layernorm(ncols, out_tag=None, affine=None):
            """Stats + normalize over partition dim (768 features, 6 chunks).

            ncols: number of valid columns (197 during init, 213 in layers).
            If affine is None: returns a new bf16 tile [128, DC, TC] (cols
            [0:ncols] valid). Else affine=(g,b) tile and the fp32 result is
            written back into xT (ln_pre).
            """
            xb = act.tile([128, DC, 2 * TC], BF16, tag="xb")
            for c in range(DC):
                nc.scalar.copy(xb[:, c, 0:ncols], xT[:, c, 0:ncols])
                nc.vector.tensor_mul(xb[:, c, ncols:2 * ncols],
                                     xb[:, c, 0:ncols], xb[:, c, 0:ncols])
            pst = pmm.tile([1, 2 * TC], F32, tag="mm")
            for c in range(DC):
                nc.tensor.matmul(pst[0:1, 0:2 * ncols], stat_lhs[:],
                                 xb[:, c, 0:2 * ncols],
                                 start=(c == 0), stop=(c == DC - 1))
            sc = small.tile([1, 2 * TC], F32, tag="sc")
            nc.vector.tensor_scalar(sc[0:1, 0:2 * ncols], pst[0:1, 0:2 * ncols],
                                    1.0 / D, None, ALU.mult)
            m2 = small.tile([1, TC], F32, tag="m2")
            nc.vector.tensor_mul(m2[0:1, 0:ncols], sc[0:1, 0:ncols], sc[0:1, 0:ncols])
            var = small.tile([1, TC], F32, tag="var")
            nc.vector.tensor_sub(var[0:1, 0:ncols], sc[0:1, ncols:2 * ncols],
                                 m2[0:1, 0:ncols])
            sd = small.tile([1, TC], F32, tag="sd")
            nc.scalar.activation(sd[0:1, 0:ncols], var[0:1, 0:ncols], AF.Sqrt,
                                 bias=eps_t[:])
            r = small.tile([1, TC], F32, tag="r")
            nc.vector.reciprocal(r[0:1, 0:ncols], sd[0:1, 0:ncols])
            mb = small.tile([1, TC], BF16, tag="mb")
            nc.vector.tensor_copy(mb[0:1, 0:ncols], sc[0:1, 0:ncols])
            rb = small.tile([1, TC], BF16, tag="rb")
            nc.vector.tensor_copy(rb[0:1, 0:ncols], r[0:1, 0:ncols])
            pm = pmm.tile([128, TC], F32, tag="mm")
            nc.tensor.matmul(pm[:, 0:ncols], ones_t[0:1, 0:128], mb[0:1, 0:ncols],
                             start=True, stop=True)
            pr = pmm.tile([128, TC], F32, tag="mm")
            nc.tensor.matmul(pr[:, 0:ncols], ones_t[0:1, 0:128], rb[0:1, 0:ncols],
                             start=True, stop=True)
            if affine is None:
                out_t = act.tile([128, DC, TC], BF16, tag=out_tag)
                for c in range(DC):
                    tmp = act.tile([128, TC], BF16, tag="lntmp")
                    nc.vector.tensor_sub(tmp[:, 0:ncols], xT[:, c, 0:ncols],
                                         pm[:, 0:ncols])
                    nc.vector.tensor_mul(out_t[:, c, 0:ncols], tmp[:, 0:ncols],
                                         pr[:, 0:ncols])
                return out_t
            g_t = affine
            for c in range(DC):
                tmp = act.tile([128, TC], F32, tag="lntmpf")
                nc.vector.tensor_sub(tmp[:, 0:ncols], xT[:, c, 0:ncols],
                                     pm[:, 0:ncols])
                nc.vector.tensor_mul(tmp[:, 0:ncols], tmp[:, 0:ncols],
                                     pr[:, 0:ncols])
                nc.vector.tensor_scalar(xT[:, c, 0:ncols], tmp[:, 0:ncols],
                                        g_t[:, c, 0:1], g_t[:, c, 1:2],
                                        ALU.mult, ALU.add)
            return None

        # ---- patch conv + pos emb + ln_pre + gv init ----
        wconv_t = []
        for k in range(DC):
            wt = wpool.tile([128, D], BF16, tag="wo")
            nc.sync.dma_start(wt[:], wconv_d[k])
            wconv_t.append(wt)
        xpat_t = []
        for k in range(DC):
            xt = pers.tile([128, T - 1], BF16, tag=f"xpat{k}")
            nc.sync.dma_start(xt[:], xpat_d[k])
            xpat_t.append(xt)
        for c in range(DC):
            post_t = act.tile([128, T], F32, tag="post")
            nc.sync.dma_start(post_t[:], post_d[c])
            pc = pmm.tile([128, TC], F32, tag="mm")
            for k in range(DC):
                nc.tensor.matmul(pc[:, 0:T - 1], wconv_t[k][:, c * 128:(c + 1) * 128],
                                 xpat_t[k][:], start=(k == 0), stop=(k == DC - 1))
            nc.vector.tensor_add(xT[:, c, 1:T], pc[:, 0:T - 1], post_t[:, 1:T])
            nc.vector.tensor_copy(xT[:, c, 0:1], post_t[:, 0:1])
        layernorm(T, affine=lnpre_t)
        for c in range(DC):
            nc.vector.tensor_copy(xT[:, c, T:TC],
                                  xT[:, c, 0:1].broadcast_to((128, NM)))

        # ---- transformer layers ----
        for li in range(L):
            # layer weights
            wqk_t, wv_t, wo_t, wfc_t = [], [], [], []
            for k in range(DC):
                wt = wpool.tile([128, QK], BF16, tag="wqk")
                nc.sync.dma_start(wt[:], wqk_d[li, k])
                wqk_t.append(wt)
            for k in range(DC):
                wt = wpool.tile([128, VW], BF16, tag="wv")
                nc.sync.dma_start(wt[:], wv_d[li, k])
                wv_t.append(wt)
            for k in range(DC):
                wt = wpool.tile([128, D], BF16, tag="wo")
                nc.sync.dma_start(wt[:], wo_d[li, k])
                wo_t.append(wt)
            bias_t = biasp.tile([1, BIASW], BF16, tag="bias")
            nc.sync.dma_start(bias_t[:], bias_d[li])

            # LN1 -> combined x|gv bf16
            xg = layernorm(TC, out_tag="xgln")

            # QK^T feature-major [1536, 213]
            qkT = act.tile([128, QKC, TC], BF16, tag="qkT")
            for m in range(QKC):
                pq = pmm.tile([128, TC], F32, tag="mm")
                for k in range(DC):
                    nc.tensor.matmul(pq[:], wqk_t[k][:, m * 128:(m + 1) * 128],
                                     xg[:, k, :], start=(k == 0), stop=False)
                nc.tensor.matmul(pq[:], bias_t[0:1, OB_QK + m * 128:OB_QK + (m + 1) * 128],
                                 ones_t[0:1, 0:TC], start=False, stop=True)
                nc.scalar.copy(qkT[:, m, :], pq[:])

            # V' token-major [213, 780] in 2 chunks (128 + 85 rows)
            v_t = act.tile([128, 2, VW], BF16, tag="vT")
            for tq, (tqs, tqn) in enumerate(((0, 128), (128, 85))):
                pvt = ppv.tile([128, VW], F32, tag="pv")
                for k in range(DC):
                    for ns, nn in ((0, 512), (512, VW - 512)):
                        nc.tensor.matmul(
                            pvt[0:tqn, ns:ns + nn],
                            xg[:, k, tqs:tqs + tqn],
                            wv_t[k][:, ns:ns + nn],
                            start=(k == 0), stop=False)
                for si, (ns, nn) in enumerate(((0, 512), (512, VW - 512))):
                    nc.tensor.matmul(
                        pvt[0:tqn, ns:ns + nn],
                        ones_t[0:1, 0:tqn],
                        bias_t[0:1, OB_V + ns:OB_V + ns + nn],
                        start=False, stop=True)
                nc.scalar.copy(v_t[0:tqn, tq, :], pvt[0:tqn, :])

            attnT = act.tile([128, DC, TC], BF16, tag="attnT")

            # ---- attention heads ----
            for h in range(NH):
                hc, hp = h // 2, (h % 2) * 64
                vs0 = v_t[0:128, 0, h * 65:(h + 1) * 65]
                vs1 = v_t[0:85, 1, h * 65:(h + 1) * 65]

                # x-stream: queries = x tokens, keys = x tokens (197)
                qs = qkT[hp:hp + 64, hc, 0:T]
                a_t = act.tile([128, 2, T], BF16, tag="aT")
                for ci, (cs, cn) in enumerate(((0, 128), (128, 69))):
                    ps_ = pmm.tile([128, TC], F32, tag="mm")
                    nc.tensor.matmul(ps_[0:cn, 0:T],
                                     qkT[hp:hp + 64, DC + hc, cs:cs + cn], qs,
                                     start=True, stop=True)
                    nc.scalar.activation(a_t[0:cn, ci, :], ps_[0:cn, 0:T], AF.Exp)
                po = pmm.tile([128, TC], F32, tag="mm")
                nc.tensor.matmul(po[0:65, 0:T], vs0, a_t[:, 0, :],
                                 start=True, stop=False)
                nc.tensor.matmul(po[0:65, 0:T], v_t[0:69, 1, h * 65:(h + 1) * 65],
                                 a_t[0:69, 1, :], start=False, stop=True)
                rd = small.tile([1, T], F32, tag="rd")
                nc.vector.reciprocal(rd[:], po[64:65, 0:T])
                rdb = small.tile([1, T], BF16, tag="rdb")
                nc.vector.tensor_copy(rdb[:], rd[:])
                pb = pmm.tile([128, TC], F32, tag="mm")
                nc.tensor.matmul(pb[0:64, 0:T], ones_t[0:1, 0:64], rdb[:],
                                 start=True, stop=True)
                ob = act.tile([64, T], BF16, tag="ob")
                nc.scalar.copy(ob[:], po[0:64, 0:T])
                nc.vector.tensor_mul(attnT[hp:hp + 64, hc, 0:T], ob[:],
                                     pb[0:64, 0:T])

                # gv stream: queries = gv cols, keys = all 213 (cls masked out)
                qg = qkT[hp:hp + 64, hc, T:TC]
                ag = act.tile([128, 2, NM], BF16, tag="ag")
                for ci, (cs, cn, mk) in enumerate(((0, 128, mk0_t), (128, 85, mk1_t))):
                    psg = pmm.tile([128, TC], F32, tag="mm")
                    nc.tensor.matmul(psg[0:cn, 0:NM],
                                     qkT[hp:hp + 64, DC + hc, cs:cs + cn], qg,
                                     start=True, stop=True)
                    nc.scalar.activation(ag[0:cn, ci, :], psg[0:cn, 0:NM], AF.Exp)
                    nc.vector.tensor_mul(ag[0:cn, ci, :], ag[0:cn, ci, :],
                                         mk[0:cn, :])
                pog = pmm.tile([128, TC], F32, tag="mm")
                nc.tensor.matmul(pog[0:65, 0:NM], vs0, ag[:, 0, :],
                                 start=True, stop=False)
                nc.tensor.matmul(pog[0:65, 0:NM], vs1, ag[0:85, 1, :],
                                 start=False, stop=True)
                rdg = small.tile([1, NM], F32, tag="rdg")
                nc.vector.reciprocal(rdg[:], pog[64:65, 0:NM])
                rdgb = small.tile([1, NM], BF16, tag="rdgb")
                nc.vector.tensor_copy(rdgb[:], rdg[:])
                pbg = pmm.tile([128, TC], F32, tag="mm")
                nc.tensor.matmul(pbg[0:64, 0:NM], ones_t[0:1, 0:64], rdgb[:],
                                 start=True, stop=True)
                obg = act.tile([64, NM], BF16, tag="obg")
                nc.scalar.copy(obg[:], pog[0:64, 0:NM])
                nc.vector.tensor_mul(attnT[hp:hp + 64, hc, T:TC], obg[:],
                                     pbg[0:64, 0:NM])

            # out projection + residual
            for c in range(DC):
                pp = pmm.tile([128, TC], F32, tag="mm")
                for k in range(DC):
                    nc.tensor.matmul(pp[:], wo_t[k][:, c * 128:(c + 1) * 128],
                                     attnT[:, k, :], start=(k == 0), stop=False)
                nc.tensor.matmul(pp[:], bias_t[0:1, OB_O + c * 128:OB_O + (c + 1) * 128],
                                 ones_t[0:1, 0:TC], start=False, stop=True)
                nc.vector.tensor_add(xT[:, c, :], xT[:, c, :], pp[:])

            # LN2 + MLP
            xg2 = layernorm(TC, out_tag="xgln")
            for k in range(DC):
                wt = wfcp.tile([128, FF], BF16, tag="wfc")
                nc.sync.dma_start(wt[:], wfc_d[li, k])
                wfc_t.append(wt)
            gT = gtp.tile([128, FFC, TC], BF16, tag="gT")
            for m in range(FFC):
                pf = pmm.tile([128, TC], F32, tag="mm")
                for k in range(DC):
                    nc.tensor.matmul(pf[:], wfc_t[k][:, m * 128:(m + 1) * 128],
                                     xg2[:, k, :], start=(k == 0), stop=False)
                nc.tensor.matmul(pf[:], bias_t[0:1, OB_FC + m * 128:OB_FC + (m + 1) * 128],
                                 ones_t[0:1, 0:TC], start=False, stop=True)
                sg = act.tile([128, TC], BF16, tag="sg")
                nc.scalar.activation(sg[:], pf[:], AF.Sigmoid, scale=1.702)
                nc.vector.tensor_mul(gT[:, m, :], pf[:], sg[:])
            wcp_t = []
            for k in range(FFC):
                wt = wpool.tile([128, D], BF16, tag="wcp")
                nc.sync.dma_start(wt[:], wcp_d[li, k])
                wcp_t.append(wt)
            for c in range(DC):
                pc = pmm.tile([128, TC], F32, tag="mm")
                for k in range(FFC):
                    nc.tensor.matmul(pc[:], wcp_t[k][:, c * 128:(c + 1) * 128],
                                     gT[:, k, :], start=(k == 0), stop=False)
                nc.tensor.matmul(pc[:], bias_t[0:1, OB_CP + c * 128:OB_CP + (c + 1) * 128],
                                 ones_t[0:1, 0:TC], start=False, stop=True)
                nc.vector.tensor_add(xT[:, c, :], xT[:, c, :], pc[:])

        # ---- final LN (affine folded into proj) + projection ----
        xgF = layernorm(TC, out_tag="xgln")
        wproj_t = []
        for k in range(DC):
            wt = wpool.tile([128, D], BF16, tag="wo")
            nc.sync.dma_start(wt[:, 0:OUT], wproj_d[k])
            wproj_t.append(wt)
        y_sb = act.tile([128, 4, NM], F32, tag="ysb")
        for c4 in range(4):
            py = pmm.tile([128, TC], F32, tag="mm")
            for k in range(DC):
                nc.tensor.matmul(py[:, 0:NM], wproj_t[k][:, c4 * 128:(c4 + 1) * 128],
                                 xgF[:, k, T:TC], start=(k == 0), stop=(k == DC - 1))
            nc.vector.tensor_copy(y_sb[:, c4, :], py[:, 0:NM])
        nc.sync.dma_start(y_d.rearrange("c p n -> p c n"), y_sb[:])


# ------------------------------------------------------------------ host side

def _prep(inputs):
    inp = {k: np.asarray(v) for k, v in inputs.items()}
    im = inp["im"].astype(np.float32)
    B = im.shape[0]
    assert B == NCORES

    # patches (conv has padding 7): [B, 196, 768] with feature order (c,kh,kw)
    imp = np.pad(im, ((0, 0), (0, 0), (7, 7), (7, 7)))[:, :, :224, :224]
    pat = imp.reshape(B, 3, 14, 16, 14, 16).transpose(0, 2, 4, 1, 3, 5)
    pat = pat.reshape(B, T - 1, D)
    wc = inp["conv_w"].astype(np.float32).reshape(D, D)

    # multiplicative attention mask M^T [213, 16] (row 0 = cls -> masked)
    masks = inp["masks"]
    inv = 1.0 - (masks != 0).astype(np.float32)
    idx = np.arange(GRID) * (masks.shape[1] // GRID)
    m14 = inv[:, idx[:, None], idx[None, :]].reshape(NM, -1)        # [16, 196]
    am = np.concatenate([m14, 1.0 - np.eye(NM, dtype=np.float32)], 1)  # [16, 212]
    M = (am == 0.0).astype(np.float32)                              # 1 = keep
    MT = np.zeros((TC, NM), np.float32)
    MT[1:, :] = M.T
    mk0 = MT[0:128].astype(BF)
    mk1 = MT[128:TC].astype(BF)

    scale = 1.0 / np.sqrt(HD)
    Wqkv = inp["qkv_w"].astype(np.float32)
    Bqkv = inp["qkv_b"].astype(np.float32)
    Wo = inp["out_w"].astype(np.float32)
    Bo = inp["out_b"].astype(np.float32)
    Wf = inp["fc_w"].astype(np.float32)
    Bf = inp["fc_b"].astype(np.float32)
    Wc = inp["cproj_w"].astype(np.float32)
    Bc = inp["cproj_b"].astype(np.float32)
    g1, b1 = inp["ln1_g"].astype(np.float32), inp["ln1_b"].astype(np.float32)
    g2, b2 = inp["ln2_g"].astype(np.float32), inp["ln2_b"].astype(np.float32)

    wqk = np.empty((L, DC, 128, QK), BF)
    wv = np.empty((L, DC, 128, VW), BF)
    wo = np.empty((L, DC, 128, D), BF)
    wfc = np.empty((L, DC, 128, FF), BF)
    wcp = np.empty((L, FFC, 128, D), BF)
    biases = np.zeros((L, 1, BIASW), BF)
    for i in range(L):
        W, b = Wqkv[i], Bqkv[i]
        Wq, bq = W[:D] * scale, b[:D] * scale
        Wk, bk = W[D:2 * D], b[D:2 * D]
        Wv_, bv_ = W[2 * D:], b[2 * D:]
        Wqk_f = (np.concatenate([Wq, Wk], 0) * g1[i][None, :]).T    # [768, 1536]
        bqk_f = np.concatenate([Wq, Wk], 0) @ b1[i] + np.concatenate([bq, bk], 0)
        Wv_f = (Wv_ * g1[i][None, :]).T                             # [768, 768]
        bv_f = Wv_ @ b1[i] + bv_
        wvp = np.zeros((D, VW), np.float32)
        bvp = np.zeros(VW, np.float32)
        for h in range(NH):
            wvp[:, h * 65:h * 65 + HD] = Wv_f[:, h * HD:(h + 1) * HD]
            bvp[h * 65:h * 65 + HD] = bv_f[h * HD:(h + 1) * HD]
            bvp[h * 65 + HD] = 1.0                                  # ones column
        wqk[i] = Wqk_f.reshape(DC, 128, QK)
        wv[i] = wvp.reshape(DC, 128, VW)
        wo[i] = Wo[i].T.reshape(DC, 128, D)
        wfc[i] = ((Wf[i] * g2[i][None, :]).T).reshape(DC, 128, FF)
        wcp[i] = Wc[i].T.reshape(FFC, 128, D)
        biases[i, 0, OB_QK:OB_QK + QK] = bqk_f
        biases[i, 0, OB_V:OB_V + VW] = bvp
        biases[i, 0, OB_O:OB_O + D] = Bo[i]
        biases[i, 0, OB_FC:OB_FC + FF] = Wf[i] @ b2[i] + Bf[i]
        biases[i, 0, OB_CP:OB_CP + D] = Bc[i]

    gp, bp = inp["ln_post_g"].astype(np.float32), inp["ln_post_b"].astype(np.float32)
    proj = inp["proj"].astype(np.float32)
    wproj = (proj * gp[:, None]).reshape(DC, 128, OUT).astype(BF)
    bproj = bp @ proj                                               # host-added

    pos = inp["pos_emb"].astype(np.float32)
    cls = inp["class_emb"].astype(np.float32)
    posT = np.concatenate([(cls + pos[0])[:, None], pos[1:].T], 1)  # [768, 197]
    post = np.ascontiguousarray(posT.reshape(DC, 128, T))
    lnpre = np.stack([inp["ln_pre_g"].astype(np.float32),
                      inp["ln_pre_b"].astype(np.float32)], 1).reshape(DC, 128, 2)
    lnpre = np.ascontiguousarray(lnpre)
    wconv = np.ascontiguousarray(wc.T.reshape(DC, 128, D)).astype(BF)

    shared = dict(post=post, lnpre=lnpre, wconv=wconv, wqk=wqk, wv=wv, wo=wo,
                  wfc=wfc, wcp=wcp, biases=biases, wproj=wproj,
                  mk0=np.ascontiguousarray(mk0), mk1=np.ascontiguousarray(mk1))
    in_maps = []
    for bidx in range(B):
        xpat = np.ascontiguousarray(pat[bidx].T.reshape(DC, 128, T - 1)).astype(BF)
        in_maps.append(dict(shared, xpat=xpat))
    return in_maps, bproj


_NC = None


def _get_nc():
    global _NC
    if _NC is None:
        _NC = _build_nc()
    return _NC


def kernel(**inputs):
    in_maps, bproj = _prep(inputs)
    nc = _get_nc()
    res = run_bass_kernel_spmd(nc, in_maps, core_ids=list(range(NCORES)))
    outs = []
    for i in range(NCORES):
        yT = res.results[i]["y"].reshape(OUT, NM).astype(np.float32)
        outs.append(yT.T + bproj[None, :])
    return np.stack(outs, 0).astype(np.float32)


# revision 15
# speedup vs baseline: 4.0261x; 4.0261x over previous
"""CLIPMaskedSpatialViT on 8 Trainium2 NeuronCores.

Strategy: data-parallel over batch (B=8 -> 1 image/core, weights replicated).
Per core the whole model runs feature-major ([D, tokens] with D on SBUF
partitions, 6 chunks of 128). The 16 per-mask global tokens (gv) are carried
as 16 extra columns next to the 197 x-tokens (213 columns total), so LN /
QKV / MLP / projections / ATTENTION for the gv stream ride in the same
matmuls as the x stream: the two attentions are one 213x213 attention with a
rectangular multiplicative mask (x queries see keys 0..196, gv queries see
everything but the cls key, per their patch masks). All matmuls are bf16 with
fp32 PSUM accumulation:
  - LN affine folded into the following matmul's weights (host-side);
    rstd computed as exp(-0.5*ln(var+eps)) so ACT stays on one LUT set
  - biases applied as K=1 matmul rows (skipped when all-zero)
  - 1/sqrt(hd) folded into Q weights
  - softmax without max-subtraction (scores are O(5)); attention denominators
    come out of the AV matmul via a ones-column appended to each head's V
  - attention masks applied multiplicatively (0/1) on exp(scores)
  - QuickGELU via the gelu_apprx_sigmoid ACT LUT (exactly x*sigmoid(1.702x))
  - all broadcasts of per-token stats via K=1 fp32 matmuls (PE)
"""
import sys

for _p in ("/opt/trn_rl_repo", "/root/.axon_site/_ro/trn_rl_repo"):
    if _p not in sys.path:
        sys.path.append(_p)

import numpy as np
import ml_dtypes

import concourse.bass as bass
import concourse.bacc as bacc
import concourse.mybir as mybir
import concourse.tile as tile
from concourse.bass_utils import run_bass_kernel_spmd

L, D, NH, FF, GRID, OUT = 12, 768, 12, 3072, 14, 512
T = GRID * GRID + 1          # 197 x tokens
NM = 16                      # masks == gv tokens
TC = T + NM                  # 213 combined columns
DC = D // 128                # 6
QK = 2 * D                   # 1536
QKC = QK // 128              # 12
FFC = FF // 128              # 24
HD = D // NH                 # 64
VW = NH * (HD + 1)           # 780 (per-head 64 V cols + 1 ones col)
NCORES = 8
EPS = 1e-5

BF = ml_dtypes.bfloat16
F32 = mybir.dt.float32
BF16 = mybir.dt.bfloat16
AF = mybir.ActivationFunctionType
ALU = mybir.AluOpType

# packed bias offsets: [bqk | bv' | bo | bfc | bcp]
OB_QK, OB_V, OB_O, OB_FC, OB_CP = 0, QK, QK + VW, QK + VW + D, QK + VW + D + FF
BIASW = QK + VW + D + FF + D   # 6924


def _build_nc(zb=True, lnid=True, gelu_lut=True):
    """zb: all non-V biases are zero -> skip their K=1 matmuls.
    lnid: ln_pre affine is identity -> skip it.
    gelu_lut: use Gelu_apprx_sigmoid LUT (HW) vs Sigmoid+mul (CoreSim)."""
    nc = bacc.Bacc("TRN2", target_bir_lowering=False, debug=False)

    xpat_d = nc.dram_tensor("xpat", [DC, 128, T - 1], BF16, kind="ExternalInput").ap()
    post_d = nc.dram_tensor("post", [DC, 128, T], F32, kind="ExternalInput").ap()
    lnpre_d = nc.dram_tensor("lnpre", [DC, 128, 2], F32, kind="ExternalInput").ap()
    wconv_d = nc.dram_tensor("wconv", [DC, 128, D], BF16, kind="ExternalInput").ap()
    wqk_d = nc.dram_tensor("wqk", [L, DC, 128, QK], BF16, kind="ExternalInput").ap()
    wv_d = nc.dram_tensor("wv", [L, DC, 128, VW], BF16, kind="ExternalInput").ap()
    wo_d = nc.dram_tensor("wo", [L, DC, 128, D], BF16, kind="ExternalInput").ap()
    wfc_d = nc.dram_tensor("wfc", [L, DC, 128, FF], BF16, kind="ExternalInput").ap()
    wcp_d = nc.dram_tensor("wcp", [L, FFC, 128, D], BF16, kind="ExternalInput").ap()
    bias_d = nc.dram_tensor("biases", [L, 1, BIASW], BF16, kind="ExternalInput").ap()
    wproj_d = nc.dram_tensor("wproj", [DC, 128, OUT], BF16, kind="ExternalInput").ap()
    mk0_d = nc.dram_tensor("mk0", [128, NM], BF16, kind="ExternalInput").ap()
    mk1_d = nc.dram_tensor("mk1", [85, NM], BF16, kind="ExternalInput").ap()
    y_d = nc.dram_tensor("y", [4, 128, NM], F32, kind="ExternalOutput").ap()

    with tile.TileContext(nc) as tc:
        _emit(nc, tc, xpat_d, post_d, lnpre_d, wconv_d, wqk_d, wv_d, wo_d,
              wfc_d, wcp_d, bias_d, wproj_d, mk0_d, mk1_d, y_d,
              zb=zb, lnid=lnid, gelu_lut=gelu_lut)
    nc.finalize()
    return nc


def _emit(nc, tc, xpat_d, post_d, lnpre_d, wconv_d, wqk_d, wv_d, wo_d,
          wfc_d, wcp_d, bias_d, wproj_d, mk0_d, mk1_d, y_d,
          zb=True, lnid=True, gelu_lut=True):
    from contextlib import ExitStack
    ctx = ExitStack()
    with ctx:
        pers = ctx.enter_context(tc.tile_pool(name="pers", bufs=1))
        act = ctx.enter_context(tc.tile_pool(name="act", bufs=2))
        ahead = ctx.enter_context(tc.tile_pool(name="ahead", bufs=6))
        wpool = ctx.enter_context(tc.tile_pool(name="w", bufs=8))
        wcpp = ctx.enter_context(tc.tile_pool(name="wcpp", bufs=16))
        wfcp = ctx.enter_context(tc.tile_pool(name="wfc", bufs=6))
        biasp = ctx.enter_context(tc.tile_pool(name="biasp", bufs=1))
        gtp = ctx.enter_context(tc.tile_pool(name="gtp", bufs=1))
        small = ctx.enter_context(tc.tile_pool(name="small", bufs=2))
        rdp = ctx.enter_context(tc.tile_pool(name="rdp", bufs=6))
        pmm = ctx.enter_context(tc.tile_pool(name="pmm", bufs=8, space="PSUM"))

        # ---- constants / persistent ----
        xT = pers.tile([128, DC, TC], F32, tag="xT")          # residual stream
        ones_t = pers.tile([1, 512], BF16, tag="ones")
        nc.vector.memset(ones_t[:], 1.0)
        onesf_t = pers.tile([1, 128], F32, tag="onesf")
        nc.vector.memset(onesf_t[:], 1.0)
        stat_lhs = pers.tile([128, 1], BF16, tag="statl")
        nc.vector.memset(stat_lhs[:], 1.0)
        eps_t = pers.tile([1, 1], F32, tag="eps")
        nc.vector.memset(eps_t[:], EPS)
        if not lnid:
            lnpre_t = pers.tile([128, DC, 2], F32, tag="lnpre")
            nc.sync.dma_start(lnpre_t[:], lnpre_d.rearrange("c p two -> p c two"))
        mk0_t = pers.tile([128, NM], BF16, tag="mk0")
        nc.sync.dma_start(mk0_t[:], mk0_d)
        mk1_t = pers.tile([85, NM], BF16, tag="mk1")
        nc.sync.dma_start(mk1_t[:], mk1_d)

        # ---- LN helper ----
        def layernorm(ncols, out_tag=None, affine=None):
            """Stats + normalize over the 768 features (6 partition chunks).

            ncols: valid columns (197 during init, 213 in layers). If affine
            is None: returns a new bf16 tile [128, DC, TC]. Else the fp32
            result is written back into xT (ln_pre path; affine may be an
            (g,b) tile or "id")."""
            xb = act.tile([128, DC, 2 * TC], BF16, tag="xb")
            for c in range(DC):
                if c % 2 == 0:
                    nc.scalar.copy(xb[:, c, 0:ncols], xT[:, c, 0:ncols])
                else:
                    nc.vector.tensor_copy(xb[:, c, 0:ncols], xT[:, c, 0:ncols])
                nc.vector.tensor_mul(xb[:, c, ncols:2 * ncols],
                                     xb[:, c, 0:ncols], xb[:, c, 0:ncols])
            pst = pmm.tile([1, 2 * TC], F32, tag="mm")
            for c in range(DC):
                nc.tensor.matmul(pst[0:1, 0:2 * ncols], stat_lhs[:],
                                 xb[:, c, 0:2 * ncols],
                                 start=(c == 0), stop=(c == DC - 1))
            sc = small.tile([1, 2 * TC], F32, tag="sc")
            nc.vector.tensor_scalar(sc[0:1, 0:2 * ncols], pst[0:1, 0:2 * ncols],
                                    1.0 / D, None, ALU.mult)
            m2 = small.tile([1, TC], F32, tag="m2")
            nc.vector.tensor_mul(m2[0:1, 0:ncols], sc[0:1, 0:ncols], sc[0:1, 0:ncols])
            var = small.tile([1, TC], F32, tag="var")
            nc.vector.tensor_sub(var[0:1, 0:ncols], sc[0:1, ncols:2 * ncols],
                                 m2[0:1, 0:ncols])
            # rstd = exp(-0.5 * ln(var + eps))  (stays on the ln/exp LUT set)
            lnv = small.tile([1, TC], F32, tag="lnv")
            nc.scalar.activation(lnv[0:1, 0:ncols], var[0:1, 0:ncols], AF.Ln,
                                 bias=eps_t[:])
            r = small.tile([1, TC], BF16, tag="r")
            nc.scalar.activation(r[0:1, 0:ncols], lnv[0:1, 0:ncols], AF.Exp,
                                 scale=-0.5)
            mb = small.tile([1, TC], BF16, tag="mb")
            nc.vector.tensor_copy(mb[0:1, 0:ncols], sc[0:1, 0:ncols])
            pm = pmm.tile([128, TC], F32, tag="mm")
            nc.tensor.matmul(pm[:, 0:ncols], ones_t[0:1, 0:128],
                             mb[0:1, 0:ncols], start=True, stop=True)
            pr = pmm.tile([128, TC], F32, tag="mm")
            nc.tensor.matmul(pr[:, 0:ncols], ones_t[0:1, 0:128],
                             r[0:1, 0:ncols], start=True, stop=True)
            if affine is None:
                out_t = act.tile([128, DC, TC], BF16, tag=out_tag)
                for c in range(DC):
                    tmp = act.tile([128, TC], BF16, tag="lntmp")
                    nc.vector.tensor_sub(tmp[:, 0:ncols], xT[:, c, 0:ncols],
                                         pm[:, 0:ncols])
                    nc.vector.tensor_mul(out_t[:, c, 0:ncols], tmp[:, 0:ncols],
                                         pr[:, 0:ncols])
                return out_t
            for c in range(DC):
                tmp = act.tile([128, TC], F32, tag="lntmpf")
                nc.vector.tensor_sub(tmp[:, 0:ncols], xT[:, c, 0:ncols],
                                     pm[:, 0:ncols])
                if affine == "id":
                    nc.vector.tensor_mul(xT[:, c, 0:ncols], tmp[:, 0:ncols],
                                         pr[:, 0:ncols])
                else:
                    nc.vector.tensor_mul(tmp[:, 0:ncols], tmp[:, 0:ncols],
                                         pr[:, 0:ncols])
                    nc.vector.tensor_scalar(xT[:, c, 0:ncols], tmp[:, 0:ncols],
                                            affine[:, c, 0:1], affine[:, c, 1:2],
                                            ALU.mult, ALU.add)
            return None

        # ---- patch conv + pos emb + ln_pre + gv init ----
        wconv_t = []
        for k in range(DC):
            wt = wpool.tile([128, D], BF16, tag="wo")
            nc.sync.dma_start(wt[:], wconv_d[k])
            wconv_t.append(wt)
        xpat_t = []
        for k in range(DC):
            xt = pers.tile([128, T - 1], BF16, tag=f"xpat{k}")
            nc.sync.dma_start(xt[:], xpat_d[k])
            xpat_t.append(xt)
        for c in range(DC):
            post_t = act.tile([128, T], F32, tag="post")
            nc.sync.dma_start(post_t[:], post_d[c])
            pc = pmm.tile([128, TC], F32, tag="mm")
            for k in range(DC):
                nc.tensor.matmul(pc[:, 0:T - 1], wconv_t[k][:, c * 128:(c + 1) * 128],
                                 xpat_t[k][:], start=(k == 0), stop=(k == DC - 1))
            nc.vector.tensor_add(xT[:, c, 1:T], pc[:, 0:T - 1], post_t[:, 1:T])
            nc.vector.tensor_copy(xT[:, c, 0:1], post_t[:, 0:1])
        layernorm(T, affine="id" if lnid else lnpre_t)
        for c in range(DC):
            nc.vector.tensor_copy(xT[:, c, T:TC],
                                  xT[:, c, 0:1].broadcast_to((128, NM)))

        # ---- transformer layers ----
        for li in range(L):
            wqk_t, wv_t, wo_t, wfc_t = [], [], [], []
            for k in range(DC):
                wt = wpool.tile([128, QK], BF16, tag="wqk")
                nc.sync.dma_start(wt[:], wqk_d[li, k])
                wqk_t.append(wt)
            for k in range(DC):
                wt = wpool.tile([128, VW], BF16, tag="wv")
                nc.sync.dma_start(wt[:], wv_d[li, k])
                wv_t.append(wt)
            for k in range(DC):
                wt = wpool.tile([128, D], BF16, tag="wo")
                nc.sync.dma_start(wt[:], wo_d[li, k])
                wo_t.append(wt)
            bias_t = biasp.tile([1, BIASW], BF16, tag="bias")
            nc.sync.dma_start(bias_t[:], bias_d[li])

            # LN1 -> combined x|gv bf16
            xg = layernorm(TC, out_tag="xgln")

            # V' token-major [213, 780] in 2 chunks (128 + 85 rows); the bias
            # row also writes the per-head ones column (denominator trick)
            v_t = act.tile([128, 2, VW], BF16, tag="vT")
            for tq, (tqs, tqn) in enumerate(((0, 128), (128, 85))):
                for ns, nn in ((0, 512), (512, VW - 512)):
                    pvt = pmm.tile([128, 512], F32, tag="mm")
                    for k in range(DC):
                        nc.tensor.matmul(
                            pvt[0:tqn, 0:nn],
                            xg[:, k, tqs:tqs + tqn],
                            wv_t[k][:, ns:ns + nn],
                            start=(k == 0), stop=False)
                    nc.tensor.matmul(
                        pvt[0:tqn, 0:nn],
                        ones_t[0:1, 0:tqn],
                        bias_t[0:1, OB_V + ns:OB_V + ns + nn],
                        start=False, stop=True)
                    nc.scalar.copy(v_t[0:tqn, tq, ns:ns + nn], pvt[0:tqn, 0:nn])

            # QK^T feature-major [1536, 213]; K chunks (6..11) first so the
            # attention scores can start before the Q-side finishes
            qkT = act.tile([128, QKC, TC], BF16, tag="qkT")
            for m in list(range(DC, QKC)) + list(range(DC)):
                pq = pmm.tile([128, TC], F32, tag="mm")
                for k in range(DC):
                    nc.tensor.matmul(pq[:], wqk_t[k][:, m * 128:(m + 1) * 128],
                                     xg[:, k, :], start=(k == 0),
                                     stop=(zb and k == DC - 1))
                if not zb:
                    nc.tensor.matmul(
                        pq[:], bias_t[0:1, OB_QK + m * 128:OB_QK + (m + 1) * 128],
                        ones_t[0:1, 0:TC], start=False, stop=True)
                if m % 2 == 0:
                    nc.scalar.copy(qkT[:, m, :], pq[:])
                else:
                    nc.vector.tensor_copy(qkT[:, m, :], pq[:])

            attnT = act.tile([128, DC, TC], BF16, tag="attnT")

            # prefetch cproj weights during the attention phase
            wcp_t = []
            for k in range(FFC):
                wt = wcpp.tile([128, D], BF16, tag="wcp")
                nc.sync.dma_start(wt[:], wcp_d[li, k])
                wcp_t.append(wt)

            # ---- merged attention: 213 queries (x|gv) x 213 keys ----
            # software-pipelined: emit head h's scores+AV (stage A), then
            # head h-1's normalize (stage B) so PE's in-order stream never
            # blocks on the DVE reciprocal.
            def attn_stage_a(h):
                hc, hp = h // 2, (h % 2) * 64
                qall = qkT[hp:hp + 64, hc, :]              # [64, 213] queries
                vs0 = v_t[0:128, 0, h * 65:(h + 1) * 65]
                vs1_85 = v_t[0:85, 1, h * 65:(h + 1) * 65]
                vs1_69 = v_t[0:69, 1, h * 65:(h + 1) * 65]
                a_t = ahead.tile([128, 2, TC], BF16, tag="aT")
                for ci, (cs, cn, mk) in enumerate(((0, 128, mk0_t), (128, 85, mk1_t))):
                    ps_ = pmm.tile([128, TC], F32, tag="mm")
                    nc.tensor.matmul(ps_[0:cn, :],
                                     qkT[hp:hp + 64, DC + hc, cs:cs + cn], qall,
                                     start=True, stop=True)
                    nc.scalar.activation(a_t[0:cn, ci, :], ps_[0:cn, :], AF.Exp)
                    nc.vector.tensor_mul(a_t[0:cn, ci, T:TC],
                                         a_t[0:cn, ci, T:TC], mk[0:cn, :])
                po = pmm.tile([128, TC], F32, tag="mm")
                # x queries: keys 0..196 only
                nc.tensor.matmul(po[0:65, 0:T], vs0, a_t[:, 0, 0:T],
                                 start=True, stop=False)
                nc.tensor.matmul(po[0:65, 0:T], vs1_69, a_t[0:69, 1, 0:T],
                                 start=False, stop=True)
                # gv queries: all 213 keys (cls + masked keys zeroed via mask)
                nc.tensor.matmul(po[0:65, T:TC], vs0, a_t[:, 0, T:TC],
                                 start=True, stop=False)
                nc.tensor.matmul(po[0:65, T:TC], vs1_85, a_t[0:85, 1, T:TC],
                                 start=False, stop=True)
                rd = rdp.tile([1, TC], BF16, tag="rd")
                with nc.allow_low_precision("softmax denominator bf16"):
                    nc.vector.reciprocal(rd[:], po[64:65, :])
                ob = ahead.tile([64, TC], BF16, tag="ob")
                if h % 2 == 0:
                    nc.scalar.copy(ob[:], po[0:64, :])
                else:
                    nc.vector.tensor_copy(ob[:], po[0:64, :])
                return h, rd, ob

            def attn_stage_b(h, rd, ob):
                hc, hp = h // 2, (h % 2) * 64
                pb = pmm.tile([128, TC], F32, tag="mm")
                nc.tensor.matmul(pb[0:64, :], ones_t[0:1, 0:64], rd[:],
                                 start=True, stop=True)
                nc.vector.tensor_mul(attnT[hp:hp + 64, hc, :], ob[:], pb[0:64, :])

            prev = None
            for h in range(NH):
                cur = attn_stage_a(h)
                if prev is not None:
                    attn_stage_b(*prev)
                prev = cur
            attn_stage_b(*prev)

            # out projection + residual
            for c in range(DC):
                pp = pmm.tile([128, TC], F32, tag="mm")
                for k in range(DC):
                    nc.tensor.matmul(pp[:], wo_t[k][:, c * 128:(c + 1) * 128],
                                     attnT[:, k, :], start=(k == 0),
                                     stop=(zb and k == DC - 1))
                if not zb:
                    nc.tensor.matmul(
                        pp[:], bias_t[0:1, OB_O + c * 128:OB_O + (c + 1) * 128],
                        ones_t[0:1, 0:TC], start=False, stop=True)
                nc.vector.tensor_add(xT[:, c, :], xT[:, c, :], pp[:])

            # LN2 + MLP
            xg2 = layernorm(TC, out_tag="xgln")
            for k in range(DC):
                wt = wfcp.tile([128, FF], BF16, tag="wfc")
                nc.sync.dma_start(wt[:], wfc_d[li, k])
                wfc_t.append(wt)
            gT = gtp.tile([128, FFC, TC], BF16, tag="gT")
            for m in range(FFC):
                pf = pmm.tile([128, TC], F32, tag="mm")
                for k in range(DC):
                    nc.tensor.matmul(pf[:], wfc_t[k][:, m * 128:(m + 1) * 128],
                                     xg2[:, k, :], start=(k == 0),
                                     stop=(zb and k == DC - 1))
                if not zb:
                    nc.tensor.matmul(
                        pf[:], bias_t[0:1, OB_FC + m * 128:OB_FC + (m + 1) * 128],
                        ones_t[0:1, 0:TC], start=False, stop=True)
                if gelu_lut:
                    nc.scalar.activation(gT[:, m, :], pf[:],
                                         AF.Gelu_apprx_sigmoid)
                else:
                    sg = act.tile([128, TC], BF16, tag="sg")
                    nc.scalar.activation(sg[:], pf[:], AF.Sigmoid, scale=1.702)
                    nc.vector.tensor_mul(gT[:, m, :], pf[:], sg[:])
            for c in range(DC):
                pc = pmm.tile([128, TC], F32, tag="mm")
                for k in range(FFC):
                    nc.tensor.matmul(pc[:], wcp_t[k][:, c * 128:(c + 1) * 128],
                                     gT[:, k, :], start=(k == 0),
                                     stop=(zb and k == FFC - 1))
                if not zb:
                    nc.tensor.matmul(
                        pc[:], bias_t[0:1, OB_CP + c * 128:OB_CP + (c + 1) * 128],
                        ones_t[0:1, 0:TC], start=False, stop=True)
                nc.vector.tensor_add(xT[:, c, :], xT[:, c, :], pc[:])

        # ---- final LN (affine folded into proj) + projection ----
        xgF = layernorm(TC, out_tag="xgln")
        wproj_t = []
        for k in range(DC):
            wt = wpool.tile([128, D], BF16, tag="wo")
            nc.sync.dma_start(wt[:, 0:OUT], wproj_d[k])
            wproj_t.append(wt)
        y_sb = act.tile([128, 4, NM], F32, tag="ysb")
        for c4 in range(4):
            py = pmm.tile([128, TC], F32, tag="mm")
            for k in range(DC):
                nc.tensor.matmul(py[:, 0:NM], wproj_t[k][:, c4 * 128:(c4 + 1) * 128],
                                 xgF[:, k, T:TC], start=(k == 0), stop=(k == DC - 1))
            nc.vector.tensor_copy(y_sb[:, c4, :], py[:, 0:NM])
        nc.sync.dma_start(y_d.rearrange("c p n -> p c n"), y_sb[:])


# ------------------------------------------------------------------ host side

def _prep(inputs):
    inp = {k: np.asarray(v) for k, v in inputs.items()}
    im = inp["im"].astype(np.float32)
    B = im.shape[0]
    assert B == NCORES

    # patches (conv has padding 7): [B, 196, 768] with feature order (c,kh,kw)
    imp = np.pad(im, ((0, 0), (0, 0), (7, 7), (7, 7)))[:, :, :224, :224]
    pat = imp.reshape(B, 3, 14, 16, 14, 16).transpose(0, 2, 4, 1, 3, 5)
    pat = pat.reshape(B, T - 1, D)
    wc = inp["conv_w"].astype(np.float32).reshape(D, D)

    # multiplicative attention mask M^T [213, 16] (row 0 = cls -> masked)
    masks = inp["masks"]
    inv = 1.0 - (masks != 0).astype(np.float32)
    idx = np.arange(GRID) * (masks.shape[1] // GRID)
    m14 = inv[:, idx[:, None], idx[None, :]].reshape(NM, -1)        # [16, 196]
    am = np.concatenate([m14, 1.0 - np.eye(NM, dtype=np.float32)], 1)  # [16, 212]
    M = (am == 0.0).astype(np.float32)                              # 1 = keep
    MT = np.zeros((TC, NM), np.float32)
    MT[1:, :] = M.T
    mk0 = MT[0:128].astype(BF)
    mk1 = MT[128:TC].astype(BF)

    scale = 1.0 / np.sqrt(HD)
    Wqkv = inp["qkv_w"].astype(np.float32)
    Bqkv = inp["qkv_b"].astype(np.float32)
    Wo = inp["out_w"].astype(np.float32)
    Bo = inp["out_b"].astype(np.float32)
    Wf = inp["fc_w"].astype(np.float32)
    Bf = inp["fc_b"].astype(np.float32)
    Wc = inp["cproj_w"].astype(np.float32)
    Bc = inp["cproj_b"].astype(np.float32)
    g1, b1 = inp["ln1_g"].astype(np.float32), inp["ln1_b"].astype(np.float32)
    g2, b2 = inp["ln2_g"].astype(np.float32), inp["ln2_b"].astype(np.float32)

    wqk = np.empty((L, DC, 128, QK), BF)
    wv = np.empty((L, DC, 128, VW), BF)
    wo = np.empty((L, DC, 128, D), BF)
    wfc = np.empty((L, DC, 128, FF), BF)
    wcp = np.empty((L, FFC, 128, D), BF)
    biases = np.zeros((L, 1, BIASW), BF)
    for i in range(L):
        W, b = Wqkv[i], Bqkv[i]
        Wq, bq = W[:D] * scale, b[:D] * scale
        Wk, bk = W[D:2 * D], b[D:2 * D]
        Wv_, bv_ = W[2 * D:], b[2 * D:]
        Wqk_f = (np.concatenate([Wq, Wk], 0) * g1[i][None, :]).T    # [768, 1536]
        bqk_f = np.concatenate([Wq, Wk], 0) @ b1[i] + np.concatenate([bq, bk], 0)
        Wv_f = (Wv_ * g1[i][None, :]).T                             # [768, 768]
        bv_f = Wv_ @ b1[i] + bv_
        wvp = np.zeros((D, VW), np.float32)
        bvp = np.zeros(VW, np.float32)
        for h in range(NH):
            wvp[:, h * 65:h * 65 + HD] = Wv_f[:, h * HD:(h + 1) * HD]
            bvp[h * 65:h * 65 + HD] = bv_f[h * HD:(h + 1) * HD]
            bvp[h * 65 + HD] = 1.0                                  # ones column
        wqk[i] = Wqk_f.reshape(DC, 128, QK)
        wv[i] = wvp.reshape(DC, 128, VW)
        wo[i] = Wo[i].T.reshape(DC, 128, D)
        wfc[i] = ((Wf[i] * g2[i][None, :]).T).reshape(DC, 128, FF)
        wcp[i] = Wc[i].T.reshape(FFC, 128, D)
        biases[i, 0, OB_QK:OB_QK + QK] = bqk_f
        biases[i, 0, OB_V:OB_V + VW] = bvp
        biases[i, 0, OB_O:OB_O + D] = Bo[i]
        biases[i, 0, OB_FC:OB_FC + FF] = Wf[i] @ b2[i] + Bf[i]
        biases[i, 0, OB_CP:OB_CP + D] = Bc[i]

    bias_f32 = biases.astype(np.float32)
    zb = bool(
        np.all(bias_f32[:, 0, OB_QK:OB_QK + QK] == 0.0)
        and np.all(bias_f32[:, 0, OB_O:] == 0.0))

    gp, bp = inp["ln_post_g"].astype(np.float32), inp["ln_post_b"].astype(np.float32)
    proj = inp["proj"].astype(np.float32)
    wproj = (proj * gp[:, None]).reshape(DC, 128, OUT).astype(BF)
    bproj = bp @ proj                                               # host-added

    pos = inp["pos_emb"].astype(np.float32)
    cls = inp["class_emb"].astype(np.float32)
    posT = np.concatenate([(cls + pos[0])[:, None], pos[1:].T], 1)  # [768, 197]
    post = np.ascontiguousarray(posT.reshape(DC, 128, T))
    lnpre_g = inp["ln_pre_g"].astype(np.float32)
    lnpre_b = inp["ln_pre_b"].astype(np.float32)
    lnid = bool(np.all(lnpre_g == 1.0) and np.all(lnpre_b == 0.0))
    lnpre = np.ascontiguousarray(
        np.stack([lnpre_g, lnpre_b], 1).reshape(DC, 128, 2))
    wconv = np.ascontiguousarray(wc.T.reshape(DC, 128, D)).astype(BF)

    shared = dict(post=post, lnpre=lnpre, wconv=wconv, wqk=wqk, wv=wv, wo=wo,
                  wfc=wfc, wcp=wcp, biases=biases, wproj=wproj,
                  mk0=np.ascontiguousarray(mk0), mk1=np.ascontiguousarray(mk1))
    in_maps = []
    for bidx in range(B):
        xpat = np.ascontiguousarray(pat[bidx].T.reshape(DC, 128, T - 1)).astype(BF)
        in_maps.append(dict(shared, xpat=xpat))
    return in_maps, bproj, zb, lnid


_NC = {}


def _get_nc(zb=True, lnid=True, gelu_lut=True):
    key = (zb, lnid, gelu_lut)
    if key not in _NC:
        _NC[key] = _build_nc(zb=zb, lnid=lnid, gelu_lut=gelu_lut)
    return _NC[key]


def kernel(**inputs):
    in_maps, bproj, zb, lnid = _prep(inputs)
    nc = _get_nc(zb=zb, lnid=lnid, gelu_lut=True)
    res = run_bass_kernel_spmd(nc, in_maps, core_ids=list(range(NCORES)))
    outs = []
    for i in range(NCORES):
        yT = res.results[i]["y"].reshape(OUT, NM).astype(np.float32)
        outs.append(yT.T + bproj[None, :])
    return np.stack(outs, 0).astype(np.float32)
